# revision 26
# baseline (speedup 1.0000x reference)
"""CosformerAttention (causal linear attention) Trainium2 Bass kernel.

Full inputs in, full output out. Shards batch*heads over 8 NeuronCores:
device d handles sample n = d//4 and heads hA = 2*(d%4), hB = hA+1.
Per device: q/k/v projections for its 2 heads (bf16 matmuls), chunked
causal linear attention with prefix-summed inter-chunk states, and a
partial output projection over its 128 local features; the host sums
the 4 per-sample partials (bf16 partials, f32 host accumulation).

v4 math layout: within-chunk (local) scores use the RAW relu'd q/k
features with a cos-difference mask (mask * cos(theta_qi - theta_ki)
is chunk-invariant), so no sin/cos scaling is needed on the k side at
all; the per-position k scaling folds into v (v_s, v_c via one
per-partition-scale DVE mul each), and cross-chunk state/prefix math
is unchanged.  q keeps its sin/cos feature scaling for the global
term.  k_t is a raw PE transpose of the relu'd k features.

Self-contained: hardcodes L=1024, N=2, E=512, H=8 from the problem spec.
"""

import sys

if "/opt/trn_rl_repo" not in sys.path:
    sys.path.insert(0, "/opt/trn_rl_repo")

import numpy as np
import ml_dtypes

BF16NP = ml_dtypes.bfloat16

import concourse.bass as bass
import concourse.tile as tile
from concourse import mybir
import concourse.bass_utils as bass_utils
from concourse.vector_clock import ScopedClock

F32 = mybir.dt.float32
BF16 = mybir.dt.bfloat16
ALU = mybir.AluOpType
ACTF = mybir.ActivationFunctionType

L, N, E, H = 1024, 2, 512, 8
D = E // H          # 64 head dim
P = 128             # partitions / chunk size
NCHUNK = L // P     # 8
NCORES = 8
EPS = 1e-6
NWARM = 11           # PE warmup matmuls (p-state ramp) during input DMA


# ---------------------------------------------------------------------------
# This walrus build allows at most ONE semaphore wait per instruction.
# (a) Tile's tail drain carries the whole global clock: split it across
#     preceding SP nops.  (b) Skip the tail barriers + semaphore clearing --
#     the Bass preamble already dma_resets + sem_clears the entire kernel
#     semaphore range at program start, so end-of-kernel cleanup is
#     redundant and costs ~10us of EVSEM butterfly.
# ---------------------------------------------------------------------------
def _patched_drain_and_barrier(self, tick_clock, wait_clock):
    nc = self.nc
    drain_inst = nc.sync.drain()
    wait_clock.add_sem_waits(
        drain_inst.ins, ScopedClock({None: tick_clock.global_clock})
    )
    waits = list(drain_inst.ins.sync_info.on_wait or [])
    if len(waits) > 1:
        drain_inst.ins.sync_info.on_wait = [waits[0]]
        SI = type(drain_inst.ins.sync_info)
        for w in waits[1:]:
            nop = nc.sync.nop()
            si = nop.ins.sync_info
            if si is None:
                nop.ins.sync_info = SI(on_wait=[w], on_update=[])
            else:
                si.on_wait = [w]
    nc.all_engine_barrier()
    popped = nc._tile_sem_poison_stack.pop()
    assert popped is self._sem_poison


tile.TileContext._drain_and_barrier = _patched_drain_and_barrier


def _split_multi_waits(nc):
    """Move excess sem waits onto preceding same-engine NoOps (engines
    execute strictly in order, so this is equivalent)."""
    k = 0
    for f in nc.m.functions:
        for bb in f.blocks:
            insts = list(bb.instructions)
            out, changed = [], False
            for inst in insts:
                si = inst.sync_info
                waits = list(si.on_wait) if (si is not None and si.on_wait) else []
                if len(waits) > 1 and "Unassigned" not in str(inst.engine):
                    for w in waits[:-1]:
                        nop = mybir.InstNoOp(name=f"wsplit-{k}", ins=[], outs=[])
                        k += 1
                        nop.engine = inst.engine
                        nop.sync_info = type(si)(on_wait=[w], on_update=[])
                        out.append(nop)
                    si.on_wait = [waits[-1]]
                    changed = True
                out.append(inst)
            if changed:
                bb.instructions = out


def _gate_dma(nc, waiter_bis, waitee_bis):
    """Make DMA `waiter` triggers wait for `waitee` DMA completions.

    Each tile-lowered InstDMACopy carries a DMAHW semaphore add (+16) that
    fires on hardware completion; compute the cumulative target value per
    semaphore in program order and append matching waits.
    """
    waitees = {id(b.ins) for b in waitee_bis}
    cum = {}
    targets = []
    for f in nc.m.functions:
        for bb in f.blocks:
            for inst in bb.instructions:
                si = inst.sync_info
                if si is None or not si.on_update:
                    continue
                for u in si.on_update:
                    if u.sync_type != "semaphore" or "DMAHW" not in (
                            u.ant_name or ""):
                        continue
                    cum[u.id] = cum.get(u.id, 0) + u.update_value
                    if id(inst) in waitees:
                        targets.append((u.id, u.ant_name, cum[u.id]))
    for b in waiter_bis:
        inst = b.ins
        si = inst.sync_info
        waits = list(si.on_wait) if (si is not None and si.on_wait) else []
        for sem_id, name, val in targets:
            waits.append(mybir.SyncWait(
                sync_type="semaphore", id=sem_id, ant_name=name,
                wait_mode="sem-ge-imm", wait_value=val, wait_reg=None))
        if si is None:
            inst.sync_info = mybir.SyncInfo(on_wait=waits, on_update=[])
        else:
            si.on_wait = waits


def bcast(ap, dims):
    """Append broadcast (step 0) free dims to an AP."""
    return bass.AP(tensor=ap.tensor, offset=ap.offset,
                   ap=list(ap.ap) + [[0, d] for d in dims])


def bcast_mid(ap, n):
    """Insert a broadcast (step 0) dim after the partition dim of a 2D AP."""
    a = list(ap.ap)
    return bass.AP(tensor=ap.tensor, offset=ap.offset,
                   ap=[a[0], [0, n]] + a[1:])


# late16 column map (bf16): ident 0:128 | mask_cos 128:256 | outw 256:768
LATE16_COLS = 768
# misc32 column map (f32): scol 0:8 | ccol 8:16 | kb 16 | qb 17 | pad
MISC32_COLS = 20


def build_program():
    nc = bass.Bass("TRN2", target_bir_lowering=False)

    # ---- DRAM I/O (host pre-packed partition-major, contiguous) ------------
    # xT: [p, tch, e, l] packed as [128, 4096] bf16
    xT_d = nc.dram_tensor("xT", [P, 2 * 4 * 512], BF16, kind="ExternalInput").ap()
    # w: [p, proj-contiguous k(4x128) | q(4x128) | v(4x128)] = [128, 1536]
    w_d = nc.dram_tensor("w", [P, 1536], BF16, kind="ExternalInput").ap()
    # sc16: [sin bcast 0:1024 | cos bcast 1024:2048] on all partitions
    sc16_d = nc.dram_tensor("sc16", [P, 2 * L], BF16, kind="ExternalInput").ap()
    late16_d = nc.dram_tensor("late16", [P, LATE16_COLS], BF16,
                              kind="ExternalInput").ap()
    misc32_d = nc.dram_tensor("misc32", [P, MISC32_COLS], F32,
                              kind="ExternalInput").ap()
    out_d = nc.dram_tensor("out", [L, E], BF16, kind="ExternalOutput").ap()

    with tile.TileContext(nc) as tc:
        persist = tc.alloc_tile_pool(name="persist", bufs=1)
        work = tc.alloc_tile_pool(name="work", bufs=3)
        small = tc.alloc_tile_pool(name="small", bufs=4)
        # PSUM budget (8 banks): ps_a 2 + ps_d2 2 + ps_ktp 2 + ps_st 2,
        # then st+ktp are released and ps_eo (tp 1 + out 2) reuses them.
        ps_d2 = tc.alloc_tile_pool(name="ps_d2", bufs=3, space="PSUM")
        ps_st = tc.alloc_tile_pool(name="ps_st", bufs=2, space="PSUM")
        ps_a = tc.alloc_tile_pool(name="ps_a", bufs=2, space="PSUM")
        ps_ktp = tc.alloc_tile_pool(name="ps_ktp", bufs=1, space="PSUM")

        # ---- persistent tiles ---------------------------------------------
        xT = persist.tile([P, 2, 4, 512], BF16, tag="xT", name="xT")
        w_all = persist.tile([P, 3, 4, 128], BF16, tag="w", name="w")
        sc16 = persist.tile([P, 2 * L], BF16, tag="sc16", name="sc16")
        late16 = persist.tile([P, LATE16_COLS], BF16, tag="l16", name="l16")
        misc32 = persist.tile([P, MISC32_COLS], F32, tag="m32", name="m32")
        warm = persist.tile([P, 512], BF16, tag="warm", name="warm")
        # raw relu'd features, both heads stacked [A 0:64 | B 64:128]
        tmpq = persist.tile([P, L], BF16, tag="tmpq", name="tmpq")
        tmpk = persist.tile([P, L], BF16, tag="tmpk", name="tmpk")
        # q/k with sin/cos feature scaling [s-part 0:64 | c-part 64:128]
        q_f = [persist.tile([P, L], BF16, tag=f"qf{h}", name=f"qf{h}")
               for h in range(2)]
        k_f = [persist.tile([P, L], BF16, tag=f"kf{h}", name=f"kf{h}")
               for h in range(2)]
        # scaled k in sequence layout: [ki, ch, head, sc, d]
        k_t = persist.tile([P, NCHUNK, 2, 2, D], BF16, tag="kt", name="kt")
        v_t = persist.tile([P, NCHUNK, 2, D + 1], BF16, tag="vt", name="vt")
        Spfx = persist.tile([P, NCHUNK, 2, D + 1], BF16, tag="spfx", name="spfx")
        attn = persist.tile([P, NCHUNK, P], BF16, tag="attn", name="attn")
        aT = persist.tile([P, NCHUNK, P], BF16, tag="aT", name="aT")
        osb = persist.tile([P, NCHUNK, E], BF16, tag="osb", name="osb")

        ident = late16[:, 0:128]
        mask = late16[:, 128:256]
        outw = late16[:, 256:768]
        kb = misc32[:, 16:17]
        qb = misc32[:, 17:18]
        scS = sc16[:, 0:L]
        scC = sc16[:, L:2 * L]

        def wsel(proj, e):  # proj: 0=k, 1=q, 2=v
            return w_all[:, proj, e, :]

        # ---- warmup, table preload, input DMAs ----------------------------
        nc.gpsimd.memset(warm[:], 0.0)
        nc.gpsimd.memset(v_t[:, :, :, D:D + 1], 1.0)

        # SP ring: xT half 0 alone, then half 1 gated on its completion so
        # the ring does not drag xT0's completion to the end of the phase.
        d_xt0 = nc.sync.dma_start(out=xT[:, 0], in_=xT_d[:, 0:2048].rearrange(
            "p (e l) -> p e l", e=4))
        d_xt1 = nc.sync.dma_start(out=xT[:, 1], in_=xT_d[:, 2048:4096].rearrange(
            "p (e l) -> p e l", e=4))
        # ACT ring: misc32 + weights first; sc16/late16 gated on w.
        nc.scalar.dma_start(out=misc32[:], in_=misc32_d)
        d_w = nc.scalar.dma_start(out=w_all[:], in_=w_d.rearrange(
            "p (j e c) -> p j e c", j=3, e=4))
        nc.scalar.dma_start(out=late16[:], in_=late16_d)
        d_sc = nc.scalar.dma_start(out=sc16[:], in_=sc16_d)

        # ACT PWP table preload (Relu) while DMAs run
        dum = work.tile([P, 8], BF16, tag="dum")
        nc.scalar.activation(dum[:], warm[:, 0:8], ACTF.Relu, scale=1.0)

        for i in range(NWARM):
            pw = ps_a.tile([P, 512], F32, tag="big")
            nc.tensor.matmul(pw[:], warm[:, 0:128], warm[:], start=True, stop=True)


        # ---- stage B-k: relu'd k, sin/cos-scaled k_f, seq-layout k_t ------
        for tch in range(2):
            cs = slice(tch * 512, (tch + 1) * 512)
            ps = ps_a.tile([P, 512], F32, tag="big")
            for e in range(4):
                nc.tensor.matmul(ps[:], wsel(0, e), xT[:, tch, e, :],
                                 start=(e == 0), stop=(e == 3))
            nc.scalar.activation(tmpk[:, cs], ps[:], ACTF.Relu, bias=kb,
                                 scale=1.0)
            nc.vector.tensor_mul(k_f[0][0:64, cs], tmpk[0:64, cs], scS[0:64, cs])
            nc.vector.tensor_mul(k_f[0][64:128, cs], tmpk[0:64, cs],
                                 scC[0:64, cs])
            nc.vector.tensor_mul(k_f[1][0:64, cs], tmpk[64:128, cs],
                                 scS[64:128, cs])
            nc.vector.tensor_mul(k_f[1][64:128, cs], tmpk[64:128, cs],
                                 scC[64:128, cs])
            # transpose this half's 4 chunks to sequence layout (scaled)
            ktp = ps_ktp.tile([P, 4, 2, P], BF16, tag="ktp")
            for sub in range(4):
                ch = 4 * tch + sub
                for h in range(2):
                    nc.tensor.transpose(ktp[:, sub, h, :],
                                        k_f[h][:, ch * P:(ch + 1) * P], ident)
            nc.vector.tensor_copy(
                k_t[:, 4 * tch:4 * tch + 4].rearrange("p c h s d -> p c h (s d)"),
                ktp[:])

        # ---- stage C: sequence-layout v (4 chunks per bank) ---------------
        def c_group(g):
            ps = ps_a.tile([P, 512], F32, tag="big")
            pv = ps.rearrange("p (s c) -> p s c", s=4)
            for sub in range(4):
                ch = 4 * g + sub
                tch, lo = ch // 4, (ch % 4) * P
                for e in range(4):
                    nc.tensor.matmul(pv[:, sub, :], xT[:, tch, e, lo:lo + P],
                                     wsel(2, e), start=(e == 0), stop=(e == 3))
            sl = slice(4 * g, 4 * g + 4)
            # plain v (ACT; ones col via memset above)
            nc.scalar.activation(
                v_t[:, sl, :, 0:D],
                pv.rearrange("p s (h d) -> p s h d", h=2), ACTF.Copy)

        # ---- stage B-q: raw q features + sin/cos-scaled q_f ----------------
        def bq(tch):
            cs = slice(tch * 512, (tch + 1) * 512)
            ps = ps_a.tile([P, 512], F32, tag="big")
            for e in range(4):
                nc.tensor.matmul(ps[:], wsel(1, e), xT[:, tch, e, :],
                                 start=(e == 0), stop=(e == 3))
            nc.scalar.activation(tmpq[:, cs], ps[:], ACTF.Relu, bias=qb,
                                 scale=1.0)

        def qmul(tch):
            cs = slice(tch * 512, (tch + 1) * 512)
            nc.vector.tensor_mul(q_f[0][0:64, cs], tmpq[0:64, cs], scS[0:64, cs])
            nc.vector.tensor_mul(q_f[0][64:128, cs], tmpq[0:64, cs],
                                 scC[0:64, cs])
            nc.vector.tensor_mul(q_f[1][0:64, cs], tmpq[64:128, cs],
                                 scS[64:128, cs])
            nc.vector.tensor_mul(q_f[1][64:128, cs], tmpq[64:128, cs],
                                 scC[64:128, cs])

        c_group(0)
        bq(0)
        qmul(0)
        c_group(1)
        bq(1)
        qmul(1)

        # ---- stage D1: per-chunk local states + prefix sum (from psum) ----
        pscs = []
        for ch in range(NCHUNK - 1):   # last chunk's state never needed
            psc = ps_st.tile([P, 2, D + 1], F32, tag="st")
            for h in range(2):
                nc.tensor.matmul(psc[:, h, :], k_t[:, ch, h, :, :],
                                 v_t[:, ch, h, :], start=True, stop=True)
            pscs.append(psc)
        nc.vector.tensor_copy(Spfx[:, 1], pscs[0][:])
        nc.vector.tensor_add(Spfx[:, 2], Spfx[:, 1], pscs[1][:])

        def prefix(ch):
            nc.vector.tensor_add(Spfx[:, ch], Spfx[:, ch - 1], pscs[ch - 1][:])
        ps_ktp.release()
        ps_a.release()
        # tp (1) + out (2) banks reuse the released ktp+a space; st pool is
        # still live (prefix adds are interleaved into D2)
        ps_eo = tc.alloc_tile_pool(name="ps_eo", bufs=1, space="PSUM")

        # ---- stage D2 + E interleaved -------------------------------------
        # D2, one bank per chunk: pssA 0:128 | pssB 128:256 | po 256:386
        def d2_chunk(ch):
            cs = slice(ch * P, (ch + 1) * P)
            d2 = ps_d2.tile([P, 386], F32, tag="d2")
            po = d2[:, 256:386].rearrange("p (h v) -> p h v", h=2)
            for h in range(2):
                nc.tensor.matmul(d2[:, h * P:(h + 1) * P], k_f[h][:, cs],
                                 q_f[h][:, cs], start=True, stop=True)
            ms = work.tile([P, 2, P], BF16, tag="ms")
            nc.vector.tensor_mul(
                ms[:], d2[:, 0:256].rearrange("p (h q) -> p h q", h=2),
                bcast_mid(mask, 2))
            for h in range(2):
                nc.tensor.matmul(po[:, h, :], ms[:, h, :], v_t[:, ch, h, :],
                                 start=True, stop=(ch == 0))
                if ch > 0:
                    nc.tensor.matmul(po[:, h, :], q_f[h][:, cs],
                                     Spfx[:, ch, h, :], start=False, stop=True)
            den = small.tile([P, 2], F32, tag="den")
            nc.vector.tensor_scalar(den[:], po[:, :, D], scalar1=EPS,
                                    scalar2=None, op0=ALU.max)
            rec = small.tile([P, 2], F32, tag="rec")
            nc.vector.reciprocal(rec[:], den[:])
            nc.vector.tensor_mul(
                attn[:, ch, :].rearrange("p (h d) -> p h d", h=2),
                po[:, :, 0:D],
                bcast(rec[:, :], [D]),
            )

        # E, groups of 4 chunks: transpose + out-proj + store (2-chunk DMAs)
        def e_group(g):
            tp = ps_eo.tile([P, 4, P], BF16, tag="tp", bufs=1)
            for i in range(4):
                nc.tensor.transpose(tp[:, i, :], attn[:, 4 * g + i, :], ident)
            nc.vector.tensor_copy(aT[:, 4 * g:4 * g + 4, :], tp[:])
            for i in range(4):
                ch = 4 * g + i
                pso = ps_eo.tile([P, E], F32, tag="out", bufs=2)
                nc.tensor.matmul(pso[:], aT[:, ch, :], outw, start=True,
                                 stop=True)
                nc.scalar.activation(osb[:, ch, :], pso[:], ACTF.Copy)
                if ch % 2 == 1:
                    nc.sync.dma_start(
                        out=out_d.rearrange(
                            "(c p) e -> p c e", p=P)[:, ch - 1:ch + 1, :],
                        in_=osb[:, ch - 1:ch + 1, :])

        for ch in range(5):
            if 3 <= ch + 2 <= 7:
                prefix(ch + 2)
            d2_chunk(ch)
        e_group(0)
        for ch in range(5, NCHUNK):
            if 3 <= ch + 2 <= 7:
                prefix(ch + 2)
            d2_chunk(ch)
        e_group(1)

        for p in (ps_eo, ps_st, ps_d2, small, work, persist):
            p.release()

    _gate_dma(nc, [d_xt1], [d_xt0])
    _gate_dma(nc, [d_sc], [d_w])
    _split_multi_waits(nc)
    return nc


_PROG = {}


def _get_program():
    if "nc" not in _PROG:
        _PROG["nc"] = build_program()
    return _PROG["nc"]


def _prep_core_inputs(dev, query, q_w, q_b, k_w, k_b, v_w, v_b, out_w):
    n = dev // 4
    hA = 2 * (dev % 4)
    aA, aB = hA * D, (hA + 1) * D

    x = np.asarray(query[:, n, :], np.float32)          # (L, E)
    xT = x.reshape(2, 512, 4, P).transpose(3, 0, 2, 1)  # (p, tch, e, l)
    xT = np.ascontiguousarray(xT.reshape(P, 4096))

    def blk(w):
        # (p, e, 128): cols = head A feats 0:64, head B feats 64:128
        b = np.concatenate([w[aA:aA + D, :], w[aB:aB + D, :]], 0).T  # (512,128)
        return b.reshape(4, P, P).transpose(1, 0, 2)

    wk = blk(np.asarray(k_w, np.float32))
    wq = blk(np.asarray(q_w, np.float32))
    wv = blk(np.asarray(v_w, np.float32))
    w_pack = np.ascontiguousarray(
        np.stack([wk, wq, wv], axis=1).reshape(P, 1536))

    idx = np.arange(1, L + 1, dtype=np.float64) * (np.pi / 2) / L
    s = np.sin(idx).astype(np.float32)
    c = np.cos(idx).astype(np.float32)
    sc16 = np.broadcast_to(np.concatenate([s, c]), (P, 2 * L))

    pi = np.arange(P)
    mask = (pi[:, None] <= pi[None, :]).astype(np.float32)
    outw = np.concatenate([out_w[:, aA:aA + D].T, out_w[:, aB:aB + D].T], 0)
    late16 = np.concatenate([np.eye(P, dtype=np.float32), mask, outw], axis=1)

    s_col = np.ascontiguousarray(s.reshape(NCHUNK, P).T)
    c_col = np.ascontiguousarray(c.reshape(NCHUNK, P).T)
    kb_col = np.concatenate([k_b[aA:aA + D], k_b[aB:aB + D]])[:, None]
    qb_col = np.concatenate([q_b[aA:aA + D], q_b[aB:aB + D]])[:, None]
    pad = np.zeros((P, MISC32_COLS - 18), np.float32)
    misc32 = np.concatenate([s_col, c_col, kb_col, qb_col, pad],
                            axis=1).astype(np.float32)

    return {
        "xT": xT.astype(BF16NP),
        "w": w_pack.astype(BF16NP),
        "sc16": np.ascontiguousarray(sc16).astype(BF16NP),
        "late16": np.ascontiguousarray(late16).astype(BF16NP),
        "misc32": np.ascontiguousarray(misc32),
    }


def run(inputs, trace=False, trace_kwargs=None):
    nc = _get_program()
    in_maps = [
        _prep_core_inputs(
            d, inputs["query"], inputs["q_w"], inputs["q_b"], inputs["k_w"],
            inputs["k_b"], inputs["v_w"], inputs["v_b"], inputs["out_w"])
        for d in range(NCORES)
    ]
    res = bass_utils.run_bass_kernel_spmd(
        nc, in_maps, list(range(NCORES)), trace=trace,
        **(trace_kwargs or {}),
    )
    parts = [res.results[i]["out"].astype(np.float32) for i in range(NCORES)]
    out0 = parts[0] + parts[1] + parts[2] + parts[3]
    out1 = parts[4] + parts[5] + parts[6] + parts[7]
    # v_b passes through attention verbatim: its out-proj image folds into
    # the output bias exactly.
    bias = (np.asarray(inputs["out_b"], np.float32)
            + np.asarray(inputs["out_w"], np.float32)
            @ np.asarray(inputs["v_b"], np.float32))
    out = np.stack([out0, out1], axis=1) + bias[None, None, :]
    return out.astype(np.float32), res


def kernel(**inputs) -> np.ndarray:
    out, _ = run(inputs, trace=False)
    return out


# revision 27
# speedup vs baseline: 1.0355x; 1.0355x over previous
"""CosformerAttention (causal linear attention) Trainium2 Bass kernel.

Full inputs in, full output out. Shards batch*heads over 8 NeuronCores:
device d handles sample n = d//4 and heads hA = 2*(d%4), hB = hA+1.
Per device: q/k/v projections for its 2 heads (bf16 matmuls), chunked
causal linear attention with prefix-summed inter-chunk states, and a
partial output projection over its 128 local features; the host sums
the 4 per-sample partials (bf16 partials, f32 host accumulation).

v4 math layout: within-chunk (local) scores use the RAW relu'd q/k
features with a cos-difference mask (mask * cos(theta_qi - theta_ki)
is chunk-invariant), so no sin/cos scaling is needed on the k side at
all; the per-position k scaling folds into v (v_s, v_c via one
per-partition-scale DVE mul each), and cross-chunk state/prefix math
is unchanged.  q keeps its sin/cos feature scaling for the global
term.  k_t is a raw PE transpose of the relu'd k features.

Self-contained: hardcodes L=1024, N=2, E=512, H=8 from the problem spec.
"""

import sys

if "/opt/trn_rl_repo" not in sys.path:
    sys.path.insert(0, "/opt/trn_rl_repo")

import numpy as np
import ml_dtypes

BF16NP = ml_dtypes.bfloat16

import concourse.bass as bass
import concourse.tile as tile
from concourse import mybir
import concourse.bass_utils as bass_utils
from concourse.vector_clock import ScopedClock

F32 = mybir.dt.float32
BF16 = mybir.dt.bfloat16
ALU = mybir.AluOpType
ACTF = mybir.ActivationFunctionType

L, N, E, H = 1024, 2, 512, 8
D = E // H          # 64 head dim
P = 128             # partitions / chunk size
NCHUNK = L // P     # 8
NCORES = 8
EPS = 1e-6
NWARM = 8           # PE warmup matmuls (p-state ramp) during input DMA


# ---------------------------------------------------------------------------
# This walrus build allows at most ONE semaphore wait per instruction.
# (a) Tile's tail drain carries the whole global clock: split it across
#     preceding SP nops.  (b) Skip the tail barriers + semaphore clearing --
#     the Bass preamble already dma_resets + sem_clears the entire kernel
#     semaphore range at program start, so end-of-kernel cleanup is
#     redundant and costs ~10us of EVSEM butterfly.
# ---------------------------------------------------------------------------
def _patched_drain_and_barrier(self, tick_clock, wait_clock):
    nc = self.nc
    drain_inst = nc.sync.drain()
    wait_clock.add_sem_waits(
        drain_inst.ins, ScopedClock({None: tick_clock.global_clock})
    )
    waits = list(drain_inst.ins.sync_info.on_wait or [])
    if len(waits) > 1:
        drain_inst.ins.sync_info.on_wait = [waits[0]]
        SI = type(drain_inst.ins.sync_info)
        for w in waits[1:]:
            nop = nc.sync.nop()
            si = nop.ins.sync_info
            if si is None:
                nop.ins.sync_info = SI(on_wait=[w], on_update=[])
            else:
                si.on_wait = [w]
    nc.all_engine_barrier()
    popped = nc._tile_sem_poison_stack.pop()
    assert popped is self._sem_poison


tile.TileContext._drain_and_barrier = _patched_drain_and_barrier


def _split_multi_waits(nc):
    """Move excess sem waits onto preceding same-engine NoOps (engines
    execute strictly in order, so this is equivalent)."""
    k = 0
    for f in nc.m.functions:
        for bb in f.blocks:
            insts = list(bb.instructions)
            out, changed = [], False
            for inst in insts:
                si = inst.sync_info
                waits = list(si.on_wait) if (si is not None and si.on_wait) else []
                if len(waits) > 1 and "Unassigned" not in str(inst.engine):
                    for w in waits[:-1]:
                        nop = mybir.InstNoOp(name=f"wsplit-{k}", ins=[], outs=[])
                        k += 1
                        nop.engine = inst.engine
                        nop.sync_info = type(si)(on_wait=[w], on_update=[])
                        out.append(nop)
                    si.on_wait = [waits[-1]]
                    changed = True
                out.append(inst)
            if changed:
                bb.instructions = out


def _gate_dma(nc, waiter_bis, waitee_bis):
    """Make DMA `waiter` triggers wait for `waitee` DMA completions.

    Each tile-lowered InstDMACopy carries a DMAHW semaphore add (+16) that
    fires on hardware completion; compute the cumulative target value per
    semaphore in program order and append matching waits.
    """
    waitees = {id(b.ins) for b in waitee_bis}
    cum = {}
    targets = []
    for f in nc.m.functions:
        for bb in f.blocks:
            for inst in bb.instructions:
                si = inst.sync_info
                if si is None or not si.on_update:
                    continue
                for u in si.on_update:
                    if u.sync_type != "semaphore" or "DMAHW" not in (
                            u.ant_name or ""):
                        continue
                    cum[u.id] = cum.get(u.id, 0) + u.update_value
                    if id(inst) in waitees:
                        targets.append((u.id, u.ant_name, cum[u.id]))
    for b in waiter_bis:
        inst = b.ins
        si = inst.sync_info
        waits = list(si.on_wait) if (si is not None and si.on_wait) else []
        for sem_id, name, val in targets:
            waits.append(mybir.SyncWait(
                sync_type="semaphore", id=sem_id, ant_name=name,
                wait_mode="sem-ge-imm", wait_value=val, wait_reg=None))
        if si is None:
            inst.sync_info = mybir.SyncInfo(on_wait=waits, on_update=[])
        else:
            si.on_wait = waits


def bcast(ap, dims):
    """Append broadcast (step 0) free dims to an AP."""
    return bass.AP(tensor=ap.tensor, offset=ap.offset,
                   ap=list(ap.ap) + [[0, d] for d in dims])


def bcast_mid(ap, n):
    """Insert a broadcast (step 0) dim after the partition dim of a 2D AP."""
    a = list(ap.ap)
    return bass.AP(tensor=ap.tensor, offset=ap.offset,
                   ap=[a[0], [0, n]] + a[1:])


# late16 column map (bf16): ident 0:128 | mask_cos 128:256 | outw 256:768
LATE16_COLS = 768
# misc32 column map (f32): scol 0:8 | ccol 8:16 | kb 16 | qb 17 | pad
MISC32_COLS = 20


def build_program():
    nc = bass.Bass("TRN2", target_bir_lowering=False)

    # ---- DRAM I/O (host pre-packed partition-major, contiguous) ------------
    # xT: [p, tch, e, l] packed as [128, 4096] bf16
    xT_d = nc.dram_tensor("xT", [P, 2 * 4 * 512], BF16, kind="ExternalInput").ap()
    # w: [p, proj-contiguous k(4x128) | q(4x128) | v(4x128)] = [128, 1536]
    w_d = nc.dram_tensor("w", [P, 1536], BF16, kind="ExternalInput").ap()
    # sc16: [sin bcast 0:1024 | cos bcast 1024:2048] on all partitions
    sc16_d = nc.dram_tensor("sc16", [P, 2 * L], BF16, kind="ExternalInput").ap()
    late16_d = nc.dram_tensor("late16", [P, LATE16_COLS], BF16,
                              kind="ExternalInput").ap()
    misc32_d = nc.dram_tensor("misc32", [P, MISC32_COLS], F32,
                              kind="ExternalInput").ap()
    out_d = nc.dram_tensor("out", [L, E], BF16, kind="ExternalOutput").ap()

    with tile.TileContext(nc) as tc:
        persist = tc.alloc_tile_pool(name="persist", bufs=1)
        work = tc.alloc_tile_pool(name="work", bufs=3)
        small = tc.alloc_tile_pool(name="small", bufs=4)
        # PSUM budget (8 banks): ps_a 2 + ps_d2 2 + ps_ktp 2 + ps_st 2,
        # then st+ktp are released and ps_eo (tp 1 + out 2) reuses them.
        ps_d2 = tc.alloc_tile_pool(name="ps_d2", bufs=3, space="PSUM")
        ps_st = tc.alloc_tile_pool(name="ps_st", bufs=2, space="PSUM")
        ps_a = tc.alloc_tile_pool(name="ps_a", bufs=2, space="PSUM")
        ps_ktp = tc.alloc_tile_pool(name="ps_ktp", bufs=1, space="PSUM")

        # ---- persistent tiles ---------------------------------------------
        xT = persist.tile([P, 2, 4, 512], BF16, tag="xT", name="xT")
        w_all = persist.tile([P, 3, 4, 128], BF16, tag="w", name="w")
        sc16 = persist.tile([P, 2 * L], BF16, tag="sc16", name="sc16")
        late16 = persist.tile([P, LATE16_COLS], BF16, tag="l16", name="l16")
        misc32 = persist.tile([P, MISC32_COLS], F32, tag="m32", name="m32")
        warm = persist.tile([P, 512], BF16, tag="warm", name="warm")
        # raw relu'd features, both heads stacked [A 0:64 | B 64:128]
        tmpq = persist.tile([P, L], BF16, tag="tmpq", name="tmpq")
        tmpk = persist.tile([P, L], BF16, tag="tmpk", name="tmpk")
        # q/k with sin/cos feature scaling [s-part 0:64 | c-part 64:128]
        q_f = [persist.tile([P, L], BF16, tag=f"qf{h}", name=f"qf{h}")
               for h in range(2)]
        k_f = [persist.tile([P, L], BF16, tag=f"kf{h}", name=f"kf{h}")
               for h in range(2)]
        # scaled k in sequence layout: [ki, ch, head, sc, d]
        k_t = persist.tile([P, NCHUNK, 2, 2, D], BF16, tag="kt", name="kt")
        v_t = persist.tile([P, NCHUNK, 2, D + 1], BF16, tag="vt", name="vt")
        Spfx = persist.tile([P, NCHUNK, 2, D + 1], BF16, tag="spfx", name="spfx")
        attn = persist.tile([P, NCHUNK, P], BF16, tag="attn", name="attn")
        aT = persist.tile([P, NCHUNK, P], BF16, tag="aT", name="aT")
        osb = persist.tile([P, NCHUNK, E], BF16, tag="osb", name="osb")

        ident = late16[:, 0:128]
        mask = late16[:, 128:256]
        outw = late16[:, 256:768]
        kb = misc32[:, 16:17]
        qb = misc32[:, 17:18]
        scS = sc16[:, 0:L]
        scC = sc16[:, L:2 * L]

        def wsel(proj, e):  # proj: 0=k, 1=q, 2=v
            return w_all[:, proj, e, :]

        # ---- warmup, table preload, input DMAs ----------------------------
        nc.gpsimd.memset(warm[:], 0.0)
        nc.gpsimd.memset(v_t[:, :, :, D:D + 1], 1.0)

        # SP ring: xT half 0 alone, then half 1 gated on its completion so
        # the ring does not drag xT0's completion to the end of the phase.
        d_xt0 = nc.sync.dma_start(out=xT[:, 0], in_=xT_d[:, 0:2048].rearrange(
            "p (e l) -> p e l", e=4))
        d_xt1 = nc.sync.dma_start(out=xT[:, 1], in_=xT_d[:, 2048:4096].rearrange(
            "p (e l) -> p e l", e=4))
        # ACT ring: misc32 + weights first; sc16/late16 gated on w.
        nc.scalar.dma_start(out=misc32[:], in_=misc32_d)
        d_w = nc.scalar.dma_start(out=w_all[:], in_=w_d.rearrange(
            "p (j e c) -> p j e c", j=3, e=4))
        d_sc = nc.scalar.dma_start(out=sc16[:], in_=sc16_d)
        nc.scalar.dma_start(out=late16[:], in_=late16_d)

        # ACT PWP table preload (Relu) while DMAs run
        dum = work.tile([P, 8], BF16, tag="dum")
        nc.scalar.activation(dum[:], warm[:, 0:8], ACTF.Relu, scale=1.0)

        for i in range(NWARM):
            pw = ps_a.tile([P, 512], F32, tag="big")
            nc.tensor.matmul(pw[:], warm[:, 0:128], warm[:], start=True, stop=True)


        # ---- stage B-k: relu'd k, sin/cos-scaled k_f, seq-layout k_t ------
        for tch in range(2):
            cs = slice(tch * 512, (tch + 1) * 512)
            ps = ps_a.tile([P, 512], F32, tag="big")
            for e in range(4):
                nc.tensor.matmul(ps[:], wsel(0, e), xT[:, tch, e, :],
                                 start=(e == 0), stop=(e == 3))
            nc.scalar.activation(tmpk[:, cs], ps[:], ACTF.Relu, bias=kb,
                                 scale=1.0)
            nc.vector.tensor_mul(k_f[0][0:64, cs], tmpk[0:64, cs], scS[0:64, cs])
            nc.vector.tensor_mul(k_f[0][64:128, cs], tmpk[0:64, cs],
                                 scC[0:64, cs])
            nc.vector.tensor_mul(k_f[1][0:64, cs], tmpk[64:128, cs],
                                 scS[64:128, cs])
            nc.vector.tensor_mul(k_f[1][64:128, cs], tmpk[64:128, cs],
                                 scC[64:128, cs])
            # transpose this half's 4 chunks to sequence layout (scaled)
            ktp = ps_ktp.tile([P, 4, 2, P], BF16, tag="ktp")
            for sub in range(4):
                ch = 4 * tch + sub
                for h in range(2):
                    nc.tensor.transpose(ktp[:, sub, h, :],
                                        k_f[h][:, ch * P:(ch + 1) * P], ident)
            nc.vector.tensor_copy(
                k_t[:, 4 * tch:4 * tch + 4].rearrange("p c h s d -> p c h (s d)"),
                ktp[:])

        # ---- stage C: sequence-layout v (4 chunks per bank) ---------------
        def c_group(g):
            ps = ps_a.tile([P, 512], F32, tag="big")
            pv = ps.rearrange("p (s c) -> p s c", s=4)
            for sub in range(4):
                ch = 4 * g + sub
                tch, lo = ch // 4, (ch % 4) * P
                for e in range(4):
                    nc.tensor.matmul(pv[:, sub, :], xT[:, tch, e, lo:lo + P],
                                     wsel(2, e), start=(e == 0), stop=(e == 3))
            sl = slice(4 * g, 4 * g + 4)
            # plain v (ACT; ones col via memset above)
            nc.scalar.activation(
                v_t[:, sl, :, 0:D],
                pv.rearrange("p s (h d) -> p s h d", h=2), ACTF.Copy)

        # ---- stage B-q: raw q features + sin/cos-scaled q_f ----------------
        def bq(tch):
            cs = slice(tch * 512, (tch + 1) * 512)
            ps = ps_a.tile([P, 512], F32, tag="big")
            for e in range(4):
                nc.tensor.matmul(ps[:], wsel(1, e), xT[:, tch, e, :],
                                 start=(e == 0), stop=(e == 3))
            nc.scalar.activation(tmpq[:, cs], ps[:], ACTF.Relu, bias=qb,
                                 scale=1.0)

        def qmul(tch):
            cs = slice(tch * 512, (tch + 1) * 512)
            nc.vector.tensor_mul(q_f[0][0:64, cs], tmpq[0:64, cs], scS[0:64, cs])
            nc.vector.tensor_mul(q_f[0][64:128, cs], tmpq[0:64, cs],
                                 scC[0:64, cs])
            nc.vector.tensor_mul(q_f[1][0:64, cs], tmpq[64:128, cs],
                                 scS[64:128, cs])
            nc.vector.tensor_mul(q_f[1][64:128, cs], tmpq[64:128, cs],
                                 scC[64:128, cs])

        c_group(0)
        bq(0)
        qmul(0)
        c_group(1)
        bq(1)
        qmul(1)

        # ---- stage D1: per-chunk local states + prefix sum (from psum) ----
        pscs = []
        for ch in range(NCHUNK - 1):   # last chunk's state never needed
            psc = ps_st.tile([P, 2, D + 1], F32, tag="st")
            for h in range(2):
                nc.tensor.matmul(psc[:, h, :], k_t[:, ch, h, :, :],
                                 v_t[:, ch, h, :], start=True, stop=True)
            pscs.append(psc)
        nc.vector.tensor_copy(Spfx[:, 1], pscs[0][:])
        nc.vector.tensor_add(Spfx[:, 2], Spfx[:, 1], pscs[1][:])

        def prefix(ch):
            nc.vector.tensor_add(Spfx[:, ch], Spfx[:, ch - 1], pscs[ch - 1][:])
        ps_ktp.release()
        ps_a.release()
        # tp (1) + out (2) banks reuse the released ktp+a space; st pool is
        # still live (prefix adds are interleaved into D2)
        ps_eo = tc.alloc_tile_pool(name="ps_eo", bufs=1, space="PSUM")

        # ---- stage D2 + E interleaved -------------------------------------
        # D2, one bank per chunk: pssA 0:128 | pssB 128:256 | po 256:386
        def d2_chunk(ch):
            cs = slice(ch * P, (ch + 1) * P)
            d2 = ps_d2.tile([P, 386], F32, tag="d2")
            po = d2[:, 256:386].rearrange("p (h v) -> p h v", h=2)
            for h in range(2):
                nc.tensor.matmul(d2[:, h * P:(h + 1) * P], k_f[h][:, cs],
                                 q_f[h][:, cs], start=True, stop=True)
            ms = work.tile([P, 2, P], BF16, tag="ms")
            nc.vector.tensor_mul(
                ms[:], d2[:, 0:256].rearrange("p (h q) -> p h q", h=2),
                bcast_mid(mask, 2))
            for h in range(2):
                nc.tensor.matmul(po[:, h, :], ms[:, h, :], v_t[:, ch, h, :],
                                 start=True, stop=(ch == 0))
                if ch > 0:
                    nc.tensor.matmul(po[:, h, :], q_f[h][:, cs],
                                     Spfx[:, ch, h, :], start=False, stop=True)
            den = small.tile([P, 2], F32, tag="den")
            nc.vector.tensor_scalar(den[:], po[:, :, D], scalar1=EPS,
                                    scalar2=None, op0=ALU.max)
            rec = small.tile([P, 2], F32, tag="rec")
            nc.vector.reciprocal(rec[:], den[:])
            nc.vector.tensor_mul(
                attn[:, ch, :].rearrange("p (h d) -> p h d", h=2),
                po[:, :, 0:D],
                bcast(rec[:, :], [D]),
            )

        # E, groups of 4 chunks: transpose + out-proj + store (2-chunk DMAs)
        def e_group(g):
            tp = ps_eo.tile([P, 4, P], BF16, tag="tp", bufs=1)
            for i in range(4):
                nc.tensor.transpose(tp[:, i, :], attn[:, 4 * g + i, :], ident)
            nc.vector.tensor_copy(aT[:, 4 * g:4 * g + 4, :], tp[:])
            for i in range(4):
                ch = 4 * g + i
                pso = ps_eo.tile([P, E], F32, tag="out", bufs=2)
                nc.tensor.matmul(pso[:], aT[:, ch, :], outw, start=True,
                                 stop=True)
                nc.scalar.activation(osb[:, ch, :], pso[:], ACTF.Copy)
                if ch % 2 == 1:
                    nc.sync.dma_start(
                        out=out_d.rearrange(
                            "(c p) e -> p c e", p=P)[:, ch - 1:ch + 1, :],
                        in_=osb[:, ch - 1:ch + 1, :])

        for ch in range(5):
            if 3 <= ch + 2 <= 7:
                prefix(ch + 2)
            d2_chunk(ch)
        e_group(0)
        for ch in range(5, NCHUNK):
            if 3 <= ch + 2 <= 7:
                prefix(ch + 2)
            d2_chunk(ch)
        e_group(1)

        for p in (ps_eo, ps_st, ps_d2, small, work, persist):
            p.release()

    _gate_dma(nc, [d_xt1], [d_xt0])
    _gate_dma(nc, [d_sc], [d_w])
    _split_multi_waits(nc)
    return nc


_PROG = {}


def _get_program():
    if "nc" not in _PROG:
        _PROG["nc"] = build_program()
    return _PROG["nc"]


def _prep_core_inputs(dev, query, q_w, q_b, k_w, k_b, v_w, v_b, out_w):
    n = dev // 4
    hA = 2 * (dev % 4)
    aA, aB = hA * D, (hA + 1) * D

    x = np.asarray(query[:, n, :], np.float32)          # (L, E)
    xT = x.reshape(2, 512, 4, P).transpose(3, 0, 2, 1)  # (p, tch, e, l)
    xT = np.ascontiguousarray(xT.reshape(P, 4096))

    def blk(w):
        # (p, e, 128): cols = head A feats 0:64, head B feats 64:128
        b = np.concatenate([w[aA:aA + D, :], w[aB:aB + D, :]], 0).T  # (512,128)
        return b.reshape(4, P, P).transpose(1, 0, 2)

    wk = blk(np.asarray(k_w, np.float32))
    wq = blk(np.asarray(q_w, np.float32))
    wv = blk(np.asarray(v_w, np.float32))
    w_pack = np.ascontiguousarray(
        np.stack([wk, wq, wv], axis=1).reshape(P, 1536))

    idx = np.arange(1, L + 1, dtype=np.float64) * (np.pi / 2) / L
    s = np.sin(idx).astype(np.float32)
    c = np.cos(idx).astype(np.float32)
    sc16 = np.broadcast_to(np.concatenate([s, c]), (P, 2 * L))

    pi = np.arange(P)
    mask = (pi[:, None] <= pi[None, :]).astype(np.float32)
    outw = np.concatenate([out_w[:, aA:aA + D].T, out_w[:, aB:aB + D].T], 0)
    late16 = np.concatenate([np.eye(P, dtype=np.float32), mask, outw], axis=1)

    s_col = np.ascontiguousarray(s.reshape(NCHUNK, P).T)
    c_col = np.ascontiguousarray(c.reshape(NCHUNK, P).T)
    kb_col = np.concatenate([k_b[aA:aA + D], k_b[aB:aB + D]])[:, None]
    qb_col = np.concatenate([q_b[aA:aA + D], q_b[aB:aB + D]])[:, None]
    pad = np.zeros((P, MISC32_COLS - 18), np.float32)
    misc32 = np.concatenate([s_col, c_col, kb_col, qb_col, pad],
                            axis=1).astype(np.float32)

    return {
        "xT": xT.astype(BF16NP),
        "w": w_pack.astype(BF16NP),
        "sc16": np.ascontiguousarray(sc16).astype(BF16NP),
        "late16": np.ascontiguousarray(late16).astype(BF16NP),
        "misc32": np.ascontiguousarray(misc32),
    }


def run(inputs, trace=False, trace_kwargs=None):
    nc = _get_program()
    in_maps = [
        _prep_core_inputs(
            d, inputs["query"], inputs["q_w"], inputs["q_b"], inputs["k_w"],
            inputs["k_b"], inputs["v_w"], inputs["v_b"], inputs["out_w"])
        for d in range(NCORES)
    ]
    res = bass_utils.run_bass_kernel_spmd(
        nc, in_maps, list(range(NCORES)), trace=trace,
        **(trace_kwargs or {}),
    )
    parts = [res.results[i]["out"].astype(np.float32) for i in range(NCORES)]
    out0 = parts[0] + parts[1] + parts[2] + parts[3]
    out1 = parts[4] + parts[5] + parts[6] + parts[7]
    # v_b passes through attention verbatim: its out-proj image folds into
    # the output bias exactly.
    bias = (np.asarray(inputs["out_b"], np.float32)
            + np.asarray(inputs["out_w"], np.float32)
            @ np.asarray(inputs["v_b"], np.float32))
    out = np.stack([out0, out1], axis=1) + bias[None, None, :]
    return out.astype(np.float32), res


def kernel(**inputs) -> np.ndarray:
    out, _ = run(inputs, trace=False)
    return out


# revision 28
# speedup vs baseline: 1.0611x; 1.0248x over previous
"""CosformerAttention (causal linear attention) Trainium2 Bass kernel.

Full inputs in, full output out. Shards batch*heads over 8 NeuronCores:
device d handles sample n = d//4 and heads hA = 2*(d%4), hB = hA+1.
Per device: q/k/v projections for its 2 heads (bf16 matmuls), chunked
causal linear attention with prefix-summed inter-chunk states, and a
partial output projection over its 128 local features; the host sums
the 4 per-sample partials (bf16 partials, f32 host accumulation).

v4 math layout: within-chunk (local) scores use the RAW relu'd q/k
features with a cos-difference mask (mask * cos(theta_qi - theta_ki)
is chunk-invariant), so no sin/cos scaling is needed on the k side at
all; the per-position k scaling folds into v (v_s, v_c via one
per-partition-scale DVE mul each), and cross-chunk state/prefix math
is unchanged.  q keeps its sin/cos feature scaling for the global
term.  k_t is a raw PE transpose of the relu'd k features.

Self-contained: hardcodes L=1024, N=2, E=512, H=8 from the problem spec.
"""

import sys

if "/opt/trn_rl_repo" not in sys.path:
    sys.path.insert(0, "/opt/trn_rl_repo")

import numpy as np
import ml_dtypes

BF16NP = ml_dtypes.bfloat16

import concourse.bass as bass
import concourse.tile as tile
from concourse import mybir
import concourse.bass_utils as bass_utils
from concourse.vector_clock import ScopedClock

F32 = mybir.dt.float32
BF16 = mybir.dt.bfloat16
ALU = mybir.AluOpType
ACTF = mybir.ActivationFunctionType

L, N, E, H = 1024, 2, 512, 8
D = E // H          # 64 head dim
P = 128             # partitions / chunk size
NCHUNK = L // P     # 8
NCORES = 8
EPS = 1e-6
NWARM = 8           # PE warmup matmuls (p-state ramp) during input DMA


# ---------------------------------------------------------------------------
# This walrus build allows at most ONE semaphore wait per instruction.
# (a) Tile's tail drain carries the whole global clock: split it across
#     preceding SP nops.  (b) Skip the tail barriers + semaphore clearing --
#     the Bass preamble already dma_resets + sem_clears the entire kernel
#     semaphore range at program start, so end-of-kernel cleanup is
#     redundant and costs ~10us of EVSEM butterfly.
# ---------------------------------------------------------------------------
def _patched_drain_and_barrier(self, tick_clock, wait_clock):
    nc = self.nc
    drain_inst = nc.sync.drain()
    wait_clock.add_sem_waits(
        drain_inst.ins, ScopedClock({None: tick_clock.global_clock})
    )
    waits = list(drain_inst.ins.sync_info.on_wait or [])
    if len(waits) > 1:
        drain_inst.ins.sync_info.on_wait = [waits[0]]
        SI = type(drain_inst.ins.sync_info)
        for w in waits[1:]:
            nop = nc.sync.nop()
            si = nop.ins.sync_info
            if si is None:
                nop.ins.sync_info = SI(on_wait=[w], on_update=[])
            else:
                si.on_wait = [w]
    nc.all_engine_barrier()
    popped = nc._tile_sem_poison_stack.pop()
    assert popped is self._sem_poison


tile.TileContext._drain_and_barrier = _patched_drain_and_barrier


def _split_multi_waits(nc):
    """Move excess sem waits onto preceding same-engine NoOps (engines
    execute strictly in order, so this is equivalent)."""
    k = 0
    for f in nc.m.functions:
        for bb in f.blocks:
            insts = list(bb.instructions)
            out, changed = [], False
            for inst in insts:
                si = inst.sync_info
                waits = list(si.on_wait) if (si is not None and si.on_wait) else []
                if len(waits) > 1 and "Unassigned" not in str(inst.engine):
                    for w in waits[:-1]:
                        nop = mybir.InstNoOp(name=f"wsplit-{k}", ins=[], outs=[])
                        k += 1
                        nop.engine = inst.engine
                        nop.sync_info = type(si)(on_wait=[w], on_update=[])
                        out.append(nop)
                    si.on_wait = [waits[-1]]
                    changed = True
                out.append(inst)
            if changed:
                bb.instructions = out


def _gate_dma(nc, waiter_bis, waitee_bis):
    """Make DMA `waiter` triggers wait for `waitee` DMA completions.

    Each tile-lowered InstDMACopy carries a DMAHW semaphore add (+16) that
    fires on hardware completion; compute the cumulative target value per
    semaphore in program order and append matching waits.
    """
    waitees = {id(b.ins) for b in waitee_bis}
    cum = {}
    targets = []
    for f in nc.m.functions:
        for bb in f.blocks:
            for inst in bb.instructions:
                si = inst.sync_info
                if si is None or not si.on_update:
                    continue
                for u in si.on_update:
                    if u.sync_type != "semaphore" or "DMAHW" not in (
                            u.ant_name or ""):
                        continue
                    cum[u.id] = cum.get(u.id, 0) + u.update_value
                    if id(inst) in waitees:
                        targets.append((u.id, u.ant_name, cum[u.id]))
    for b in waiter_bis:
        inst = b.ins
        si = inst.sync_info
        waits = list(si.on_wait) if (si is not None and si.on_wait) else []
        for sem_id, name, val in targets:
            waits.append(mybir.SyncWait(
                sync_type="semaphore", id=sem_id, ant_name=name,
                wait_mode="sem-ge-imm", wait_value=val, wait_reg=None))
        if si is None:
            inst.sync_info = mybir.SyncInfo(on_wait=waits, on_update=[])
        else:
            si.on_wait = waits


def bcast(ap, dims):
    """Append broadcast (step 0) free dims to an AP."""
    return bass.AP(tensor=ap.tensor, offset=ap.offset,
                   ap=list(ap.ap) + [[0, d] for d in dims])


def bcast_mid(ap, n):
    """Insert a broadcast (step 0) dim after the partition dim of a 2D AP."""
    a = list(ap.ap)
    return bass.AP(tensor=ap.tensor, offset=ap.offset,
                   ap=[a[0], [0, n]] + a[1:])


# late16 column map (bf16): ident 0:128 | mask_cos 128:256 | outw 256:768
LATE16_COLS = 768
# misc32 column map (f32): scol 0:8 | ccol 8:16 | kb 16 | qb 17 | pad
MISC32_COLS = 20


def build_program():
    nc = bass.Bass("TRN2", target_bir_lowering=False)

    # ---- DRAM I/O (host pre-packed partition-major, contiguous) ------------
    # xT: [p, tch, e, l] packed as [128, 4096] bf16
    xT_d = nc.dram_tensor("xT", [P, 2 * 4 * 512], BF16, kind="ExternalInput").ap()
    # w: [p, proj-contiguous k(4x128) | q(4x128) | v(4x128)] = [128, 1536]
    w_d = nc.dram_tensor("w", [P, 1536], BF16, kind="ExternalInput").ap()
    # sc16: [sin bcast 0:1024 | cos bcast 1024:2048] on all partitions
    sc16_d = nc.dram_tensor("sc16", [P, 2 * L], BF16, kind="ExternalInput").ap()
    ident_d = nc.dram_tensor("ident16", [P, P], BF16, kind="ExternalInput").ap()
    late16_d = nc.dram_tensor("late16", [P, LATE16_COLS], BF16,
                              kind="ExternalInput").ap()
    misc32_d = nc.dram_tensor("misc32", [P, MISC32_COLS], F32,
                              kind="ExternalInput").ap()
    out_d = nc.dram_tensor("out", [L, E], BF16, kind="ExternalOutput").ap()

    with tile.TileContext(nc) as tc:
        persist = tc.alloc_tile_pool(name="persist", bufs=1)
        work = tc.alloc_tile_pool(name="work", bufs=3)
        small = tc.alloc_tile_pool(name="small", bufs=4)
        # PSUM budget (8 banks): ps_a 2 + ps_d2 2 + ps_ktp 2 + ps_st 2,
        # then st+ktp are released and ps_eo (tp 1 + out 2) reuses them.
        ps_d2 = tc.alloc_tile_pool(name="ps_d2", bufs=3, space="PSUM")
        ps_st = tc.alloc_tile_pool(name="ps_st", bufs=2, space="PSUM")
        ps_a = tc.alloc_tile_pool(name="ps_a", bufs=2, space="PSUM")
        ps_ktp = tc.alloc_tile_pool(name="ps_ktp", bufs=1, space="PSUM")

        # ---- persistent tiles ---------------------------------------------
        xT = persist.tile([P, 2, 4, 512], BF16, tag="xT", name="xT")
        w_all = persist.tile([P, 3, 4, 128], BF16, tag="w", name="w")
        sc16 = persist.tile([P, 2 * L], BF16, tag="sc16", name="sc16")
        ident16 = persist.tile([P, P], BF16, tag="i16", name="i16")
        late16 = persist.tile([P, LATE16_COLS], BF16, tag="l16", name="l16")
        misc32 = persist.tile([P, MISC32_COLS], F32, tag="m32", name="m32")
        warm = persist.tile([P, 512], BF16, tag="warm", name="warm")
        # raw relu'd features, both heads stacked [A 0:64 | B 64:128]
        tmpq = persist.tile([P, L], BF16, tag="tmpq", name="tmpq")
        tmpk = persist.tile([P, L], BF16, tag="tmpk", name="tmpk")
        # q/k with sin/cos feature scaling [s-part 0:64 | c-part 64:128]
        q_f = [persist.tile([P, L], BF16, tag=f"qf{h}", name=f"qf{h}")
               for h in range(2)]
        k_f = [persist.tile([P, L], BF16, tag=f"kf{h}", name=f"kf{h}")
               for h in range(2)]
        # scaled k in sequence layout: [ki, ch, head, sc, d]
        k_t = persist.tile([P, NCHUNK, 2, 2, D], BF16, tag="kt", name="kt")
        v_t = persist.tile([P, NCHUNK, 2, D + 1], BF16, tag="vt", name="vt")
        Spfx = persist.tile([P, NCHUNK, 2, D + 1], BF16, tag="spfx", name="spfx")
        attn = persist.tile([P, NCHUNK, P], BF16, tag="attn", name="attn")
        aT = persist.tile([P, NCHUNK, P], BF16, tag="aT", name="aT")
        osb = persist.tile([P, NCHUNK, E], BF16, tag="osb", name="osb")

        ident = ident16[:]
        mask = late16[:, 128:256]
        outw = late16[:, 256:768]
        kb = misc32[:, 16:17]
        qb = misc32[:, 17:18]
        scS = sc16[:, 0:L]
        scC = sc16[:, L:2 * L]

        def wsel(proj, e):  # proj: 0=k, 1=q, 2=v
            return w_all[:, proj, e, :]

        # ---- warmup, table preload, input DMAs ----------------------------
        nc.gpsimd.memset(warm[:], 0.0)
        nc.gpsimd.memset(v_t[:, :, :, D:D + 1], 1.0)

        # SP ring: xT half 0 alone, then half 1 gated on its completion so
        # the ring does not drag xT0's completion to the end of the phase.
        d_xt0 = nc.sync.dma_start(out=xT[:, 0], in_=xT_d[:, 0:2048].rearrange(
            "p (e l) -> p e l", e=4))
        d_xt1 = nc.sync.dma_start(out=xT[:, 1], in_=xT_d[:, 2048:4096].rearrange(
            "p (e l) -> p e l", e=4))
        # ACT ring: misc32 + weights first; sc16/late16 gated on w.
        nc.scalar.dma_start(out=misc32[:], in_=misc32_d)
        nc.scalar.dma_start(out=ident16[:], in_=ident_d)
        d_w = nc.scalar.dma_start(out=w_all[:], in_=w_d.rearrange(
            "p (j e c) -> p j e c", j=3, e=4))
        d_sc = nc.scalar.dma_start(out=sc16[:], in_=sc16_d)
        nc.scalar.dma_start(out=late16[:], in_=late16_d)

        # ACT PWP table preload (Relu) while DMAs run
        dum = work.tile([P, 8], BF16, tag="dum")
        nc.scalar.activation(dum[:], warm[:, 0:8], ACTF.Relu, scale=1.0)

        for i in range(NWARM):
            pw = ps_a.tile([P, 512], F32, tag="big")
            nc.tensor.matmul(pw[:], warm[:, 0:128], warm[:], start=True, stop=True)


        # ---- stage B-k: relu'd k, sin/cos-scaled k_f, seq-layout k_t ------
        for tch in range(2):
            cs = slice(tch * 512, (tch + 1) * 512)
            ps = ps_a.tile([P, 512], F32, tag="big")
            for e in range(4):
                nc.tensor.matmul(ps[:], wsel(0, e), xT[:, tch, e, :],
                                 start=(e == 0), stop=(e == 3))
            nc.scalar.activation(tmpk[:, cs], ps[:], ACTF.Relu, bias=kb,
                                 scale=1.0)
            nc.vector.tensor_mul(k_f[0][0:64, cs], tmpk[0:64, cs], scS[0:64, cs])
            nc.vector.tensor_mul(k_f[0][64:128, cs], tmpk[0:64, cs],
                                 scC[0:64, cs])
            nc.vector.tensor_mul(k_f[1][0:64, cs], tmpk[64:128, cs],
                                 scS[64:128, cs])
            nc.vector.tensor_mul(k_f[1][64:128, cs], tmpk[64:128, cs],
                                 scC[64:128, cs])
            # transpose this half's 4 chunks to sequence layout (scaled)
            ktp = ps_ktp.tile([P, 4, 2, P], BF16, tag="ktp")
            for sub in range(4):
                ch = 4 * tch + sub
                for h in range(2):
                    nc.tensor.transpose(ktp[:, sub, h, :],
                                        k_f[h][:, ch * P:(ch + 1) * P], ident)
            nc.vector.tensor_copy(
                k_t[:, 4 * tch:4 * tch + 4].rearrange("p c h s d -> p c h (s d)"),
                ktp[:])

        # ---- stage C: sequence-layout v (4 chunks per bank) ---------------
        def c_group(g):
            ps = ps_a.tile([P, 512], F32, tag="big")
            pv = ps.rearrange("p (s c) -> p s c", s=4)
            for sub in range(4):
                ch = 4 * g + sub
                tch, lo = ch // 4, (ch % 4) * P
                for e in range(4):
                    nc.tensor.matmul(pv[:, sub, :], xT[:, tch, e, lo:lo + P],
                                     wsel(2, e), start=(e == 0), stop=(e == 3))
            sl = slice(4 * g, 4 * g + 4)
            # plain v (ACT; ones col via memset above)
            nc.scalar.activation(
                v_t[:, sl, :, 0:D],
                pv.rearrange("p s (h d) -> p s h d", h=2), ACTF.Copy)

        # ---- stage B-q: raw q features + sin/cos-scaled q_f ----------------
        def bq(tch):
            cs = slice(tch * 512, (tch + 1) * 512)
            ps = ps_a.tile([P, 512], F32, tag="big")
            for e in range(4):
                nc.tensor.matmul(ps[:], wsel(1, e), xT[:, tch, e, :],
                                 start=(e == 0), stop=(e == 3))
            nc.scalar.activation(tmpq[:, cs], ps[:], ACTF.Relu, bias=qb,
                                 scale=1.0)

        def qmul(tch):
            cs = slice(tch * 512, (tch + 1) * 512)
            nc.vector.tensor_mul(q_f[0][0:64, cs], tmpq[0:64, cs], scS[0:64, cs])
            nc.vector.tensor_mul(q_f[0][64:128, cs], tmpq[0:64, cs],
                                 scC[0:64, cs])
            nc.vector.tensor_mul(q_f[1][0:64, cs], tmpq[64:128, cs],
                                 scS[64:128, cs])
            nc.vector.tensor_mul(q_f[1][64:128, cs], tmpq[64:128, cs],
                                 scC[64:128, cs])

        c_group(0)
        bq(0)
        qmul(0)
        c_group(1)
        bq(1)
        qmul(1)

        # ---- stage D1: per-chunk local states + prefix sum (from psum) ----
        pscs = []
        for ch in range(NCHUNK - 1):   # last chunk's state never needed
            psc = ps_st.tile([P, 2, D + 1], F32, tag="st")
            for h in range(2):
                nc.tensor.matmul(psc[:, h, :], k_t[:, ch, h, :, :],
                                 v_t[:, ch, h, :], start=True, stop=True)
            pscs.append(psc)
        nc.vector.tensor_copy(Spfx[:, 1], pscs[0][:])
        nc.vector.tensor_add(Spfx[:, 2], Spfx[:, 1], pscs[1][:])

        def prefix(ch):
            nc.vector.tensor_add(Spfx[:, ch], Spfx[:, ch - 1], pscs[ch - 1][:])
        ps_ktp.release()
        ps_a.release()
        # tp (1) + out (2) banks reuse the released ktp+a space; st pool is
        # still live (prefix adds are interleaved into D2)
        ps_eo = tc.alloc_tile_pool(name="ps_eo", bufs=1, space="PSUM")

        # ---- stage D2 + E interleaved -------------------------------------
        # D2, one bank per chunk: pssA 0:128 | pssB 128:256 | po 256:386
        def d2_chunk(ch):
            cs = slice(ch * P, (ch + 1) * P)
            d2 = ps_d2.tile([P, 386], F32, tag="d2")
            po = d2[:, 256:386].rearrange("p (h v) -> p h v", h=2)
            for h in range(2):
                nc.tensor.matmul(d2[:, h * P:(h + 1) * P], k_f[h][:, cs],
                                 q_f[h][:, cs], start=True, stop=True)
            ms = work.tile([P, 2, P], BF16, tag="ms")
            nc.vector.tensor_mul(
                ms[:], d2[:, 0:256].rearrange("p (h q) -> p h q", h=2),
                bcast_mid(mask, 2))
            for h in range(2):
                nc.tensor.matmul(po[:, h, :], ms[:, h, :], v_t[:, ch, h, :],
                                 start=True, stop=(ch == 0))
                if ch > 0:
                    nc.tensor.matmul(po[:, h, :], q_f[h][:, cs],
                                     Spfx[:, ch, h, :], start=False, stop=True)
            den = small.tile([P, 2], F32, tag="den")
            nc.vector.tensor_scalar(den[:], po[:, :, D], scalar1=EPS,
                                    scalar2=None, op0=ALU.max)
            rec = small.tile([P, 2], F32, tag="rec")
            nc.vector.reciprocal(rec[:], den[:])
            nc.vector.tensor_mul(
                attn[:, ch, :].rearrange("p (h d) -> p h d", h=2),
                po[:, :, 0:D],
                bcast(rec[:, :], [D]),
            )

        # E, groups of 4 chunks: transpose + out-proj + store (2-chunk DMAs)
        def e_group(g):
            tp = ps_eo.tile([P, 4, P], BF16, tag="tp", bufs=1)
            for i in range(4):
                nc.tensor.transpose(tp[:, i, :], attn[:, 4 * g + i, :], ident)
            nc.vector.tensor_copy(aT[:, 4 * g:4 * g + 4, :], tp[:])
            for i in range(4):
                ch = 4 * g + i
                pso = ps_eo.tile([P, E], F32, tag="out", bufs=2)
                nc.tensor.matmul(pso[:], aT[:, ch, :], outw, start=True,
                                 stop=True)
                nc.scalar.activation(osb[:, ch, :], pso[:], ACTF.Copy)
                if ch % 2 == 1:
                    nc.sync.dma_start(
                        out=out_d.rearrange(
                            "(c p) e -> p c e", p=P)[:, ch - 1:ch + 1, :],
                        in_=osb[:, ch - 1:ch + 1, :])

        for ch in range(5):
            if 3 <= ch + 2 <= 7:
                prefix(ch + 2)
            d2_chunk(ch)
        e_group(0)
        for ch in range(5, NCHUNK):
            if 3 <= ch + 2 <= 7:
                prefix(ch + 2)
            d2_chunk(ch)
        e_group(1)

        for p in (ps_eo, ps_st, ps_d2, small, work, persist):
            p.release()

    _gate_dma(nc, [d_xt1], [d_xt0])
    _gate_dma(nc, [d_sc], [d_w])
    _split_multi_waits(nc)
    return nc


_PROG = {}


def _get_program():
    if "nc" not in _PROG:
        _PROG["nc"] = build_program()
    return _PROG["nc"]


def _prep_core_inputs(dev, query, q_w, q_b, k_w, k_b, v_w, v_b, out_w):
    n = dev // 4
    hA = 2 * (dev % 4)
    aA, aB = hA * D, (hA + 1) * D

    x = np.asarray(query[:, n, :], np.float32)          # (L, E)
    xT = x.reshape(2, 512, 4, P).transpose(3, 0, 2, 1)  # (p, tch, e, l)
    xT = np.ascontiguousarray(xT.reshape(P, 4096))

    def blk(w):
        # (p, e, 128): cols = head A feats 0:64, head B feats 64:128
        b = np.concatenate([w[aA:aA + D, :], w[aB:aB + D, :]], 0).T  # (512,128)
        return b.reshape(4, P, P).transpose(1, 0, 2)

    wk = blk(np.asarray(k_w, np.float32))
    wq = blk(np.asarray(q_w, np.float32))
    wv = blk(np.asarray(v_w, np.float32))
    w_pack = np.ascontiguousarray(
        np.stack([wk, wq, wv], axis=1).reshape(P, 1536))

    idx = np.arange(1, L + 1, dtype=np.float64) * (np.pi / 2) / L
    s = np.sin(idx).astype(np.float32)
    c = np.cos(idx).astype(np.float32)
    sc16 = np.broadcast_to(np.concatenate([s, c]), (P, 2 * L))

    pi = np.arange(P)
    mask = (pi[:, None] <= pi[None, :]).astype(np.float32)
    outw = np.concatenate([out_w[:, aA:aA + D].T, out_w[:, aB:aB + D].T], 0)
    late16 = np.concatenate([np.eye(P, dtype=np.float32), mask, outw], axis=1)
    ident16 = np.eye(P, dtype=np.float32)

    s_col = np.ascontiguousarray(s.reshape(NCHUNK, P).T)
    c_col = np.ascontiguousarray(c.reshape(NCHUNK, P).T)
    kb_col = np.concatenate([k_b[aA:aA + D], k_b[aB:aB + D]])[:, None]
    qb_col = np.concatenate([q_b[aA:aA + D], q_b[aB:aB + D]])[:, None]
    pad = np.zeros((P, MISC32_COLS - 18), np.float32)
    misc32 = np.concatenate([s_col, c_col, kb_col, qb_col, pad],
                            axis=1).astype(np.float32)

    return {
        "xT": xT.astype(BF16NP),
        "w": w_pack.astype(BF16NP),
        "sc16": np.ascontiguousarray(sc16).astype(BF16NP),
        "ident16": np.ascontiguousarray(ident16).astype(BF16NP),
        "late16": np.ascontiguousarray(late16).astype(BF16NP),
        "misc32": np.ascontiguousarray(misc32),
    }


def run(inputs, trace=False, trace_kwargs=None):
    nc = _get_program()
    in_maps = [
        _prep_core_inputs(
            d, inputs["query"], inputs["q_w"], inputs["q_b"], inputs["k_w"],
            inputs["k_b"], inputs["v_w"], inputs["v_b"], inputs["out_w"])
        for d in range(NCORES)
    ]
    res = bass_utils.run_bass_kernel_spmd(
        nc, in_maps, list(range(NCORES)), trace=trace,
        **(trace_kwargs or {}),
    )
    parts = [res.results[i]["out"].astype(np.float32) for i in range(NCORES)]
    out0 = parts[0] + parts[1] + parts[2] + parts[3]
    out1 = parts[4] + parts[5] + parts[6] + parts[7]
    # v_b passes through attention verbatim: its out-proj image folds into
    # the output bias exactly.
    bias = (np.asarray(inputs["out_b"], np.float32)
            + np.asarray(inputs["out_w"], np.float32)
            @ np.asarray(inputs["v_b"], np.float32))
    out = np.stack([out0, out1], axis=1) + bias[None, None, :]
    return out.astype(np.float32), res


def kernel(**inputs) -> np.ndarray:
    out, _ = run(inputs, trace=False)
    return out
